# revision 23
# baseline (speedup 1.0000x reference)
"""Embedding lookup (nn_CustomEmbedding) on 8 Trainium2 NeuronCores.

reference: out[b, t, :] = weight.T[index[b, t], :]
  index:  [4096, 200] int32/int64  (values in [0, 100000))
  weight: [128, 100000] f32
  out:    [4096, 200, 128] f32

Strategy (data-parallel batch shard, replicated bf16 table, single-pass
PAIR dma_gather on 4 SWDGE queues, raw-bass manual-semaphore pipeline):
  - Host: table -> bf16, viewed as 50000 PAIRS of rows (1 pair = 2 rows
    = 512B). ONE descriptor per lookup fetches the pair containing the
    row: idx = (v>>1) - 32768 (signed int16 spans 65536 pairs = 131072
    rows >= 100000), elem_size = elem_step = 512B.
  - On-chip, the wanted half of each pair is chosen by v&1: DVE
    copy_predicated (uint8 mask broadcast along the 128-elem row)
    overwrites the low half with the high half where v is odd.
  - The device stores bf16; the HOST upconverts to f32 (halves store
    traffic and removes the cast from the device critical path).
  - 4 SWDGE queues: the gather ucode runs each instruction's desc-gen on
    the Q7 core pair selected by queue_num (cpu_id/2 == queue_num), so 4
    queues = 4 fully parallel desc-gen streams. The TileContext version
    serialized desc-gen behind DMA-completion bridge events on Pool;
    here all cross-engine deps are manual sems waited on the CONSUMER:
      Pool:  back-to-back dma_gather (queue g%4) -> dma_sem[g%8] += 16
      DVE :  wait dma_sem        -> copy_predicated -> pred_sem += 1
      Act :  wait pred_sem -> copy lo into a CONTIGUOUS bf16 tile
             (a strided store source would shatter the HWDGE store into
             256B descriptors) -> act_sem += 1
      SP  :  idx/msk loads up front; wait act_sem -> contiguous store
             (HWDGE) -> store_sem += 16;  Pool reuses a dst tile after
             the Act copy of the group NBUF back (act_sem).
  - rel-err from bf16 rounding <= 2^-9, far inside the 2e-2 gate.
"""

from contextlib import ExitStack

import numpy as np

import concourse.bacc as bacc
import concourse.mybir as mybir
from concourse.bass_utils import run_bass_kernel_spmd
from concourse.library_config import mlp

V = 100000
D = 128
N_CORES = 8
N_TOTAL = 4096 * 200  # 819200
N_CORE = N_TOTAL // N_CORES  # 102400
# group sizes tapered: small groups at both ends prime the pipeline
# faster (first DVE/store starts sooner) and shrink the post-desc-gen
# DMA drain tail; +16 trailing nonnegative dummies per group (the gather
# ucode trims trailing-negative idxs, which would leave garbage slots)
SIZES = [1664] * 4 + [3200] * 28 + [1536] * 4  # multiples of 128, sum N_CORE
assert sum(SIZES) == N_CORE
NG = len(SIZES)
OFFS = [sum(SIZES[:g]) for g in range(NG)]  # position offset per group
NCOLS_G = [ni // 128 for ni in SIZES]  # gathered pairs per partition
ICOLS_G = [(ni + 16) // 16 for ni in SIZES]  # idx stripe columns
IOFF = [sum(ICOLS_G[:g]) for g in range(NG + 1)]
MOFF = [sum(NCOLS_G[:g]) for g in range(NG + 1)]
NDST = (max(SIZES) + 16) // 128 + 1  # 26 pair columns in the dst tile
N_QUEUES = 4
NBUF = 8  # gather dst tiles in flight
NOUT = 4  # contiguous store-staging tiles
N_DMA_SEMS = 8

_cached = {}


def _build():
    nc = bacc.Bacc(
        "TRN2",
        target_bir_lowering=False,
        debug=False,
        enable_asserts=False,
        num_devices=N_CORES,
        num_swdge_queues=N_QUEUES,
        dynamic_dma_scratch_size=32768,
    )
    idx_dram = nc.dram_tensor(
        "idx", [128, IOFF[NG]], mybir.dt.int16, kind="ExternalInput"
    )
    msk_dram = nc.dram_tensor(
        "msk", [128, MOFF[NG]], mybir.dt.uint8, kind="ExternalInput"
    )
    ext_dram = nc.dram_tensor("ext", [V, D], mybir.dt.bfloat16, kind="ExternalInput")
    out_dram = nc.dram_tensor(
        "out", [N_CORE, D], mybir.dt.bfloat16, kind="ExternalOutput"
    )

    # pair view: entry a = table rows [2a, 2a+1] (512B), based mid-window at
    # pair 32768 (row 65536) so signed int16 indices reach all 50000 pairs
    pair_view = ext_dram.ap()[65536:V].rearrange("(a two) d -> a (two d)", two=2)

    # out group g, partition p holds positions OFFS[g] + p*NCOLS_G[g] ...
    def out_view(g):
        return (
            out_dram.ap()[OFFS[g] : OFFS[g] + SIZES[g]]
            .rearrange("(p c) d -> p (c d)", p=128, c=NCOLS_G[g])
        )

    with (
        nc.Block(no_gpsimd_drain=True) as block,
        nc.sbuf_tensor("idx_sb", [128, IOFF[NG]], mybir.dt.int16) as idx_sb,
        nc.sbuf_tensor("msk_sb", [128, MOFF[NG]], mybir.dt.uint8) as msk_sb,
        nc.sbuf_tensor(
            "dst_sb", [128, NBUF, NDST, 2 * D], mybir.dt.bfloat16
        ) as dst_sb,
        nc.sbuf_tensor(
            "cont_sb", [128, NOUT, (max(SIZES) // 128) * D], mybir.dt.bfloat16
        ) as cont_sb,
        nc.semaphore("ld") as ld_sem,
        nc.semaphore("pred") as pred_sem,
        nc.semaphore("act") as act_sem,
        nc.semaphore("store") as store_sem,
        ExitStack() as stack,
    ):
        dma_sems = [
            stack.enter_context(nc.semaphore(f"dma{i}")) for i in range(N_DMA_SEMS)
        ]

        def lo_hi_mexp(g):
            b = g % NBUF
            nc_g = NCOLS_G[g]
            lo = dst_sb[:, b, :nc_g, 0:D]
            hi = dst_sb[:, b, :nc_g, D : 2 * D]
            mexp = msk_sb[:, MOFF[g] : MOFF[g + 1]].broadcast_to(
                [128, nc_g, D]
            )
            return lo, hi, mexp

        @block.sync
        def _(sp):
            # idx slice 0 first so gather 0 can start ASAP, then msk,
            # then the remaining idx slices: slice g ready at ld>=16*(g+2)
            sp.dma_start(
                idx_sb[:, IOFF[0] : IOFF[1]], idx_dram.ap()[:, IOFF[0] : IOFF[1]]
            ).then_inc(ld_sem, 16)
            sp.dma_start(msk_sb[:], msk_dram.ap()).then_inc(ld_sem, 16)
            for g in range(1, NG):
                sp.dma_start(
                    idx_sb[:, IOFF[g] : IOFF[g + 1]],
                    idx_dram.ap()[:, IOFF[g] : IOFF[g + 1]],
                ).then_inc(ld_sem, 16)
            for g in range(NG):
                sp.wait_ge(act_sem, g + 1)
                sp.dma_start(
                    out_view(g), cont_sb[:, g % NOUT, : SIZES[g] * D // 128]
                ).then_inc(store_sem, 16)
            sp.wait_ge(store_sem, 16 * NG)

        @block.gpsimd
        def _(gp):
            gp.load_library(mlp)
            for g in range(NG):
                if g >= NBUF:
                    gp.wait_ge(act_sem, g - NBUF + 1)
                gp.wait_ge(ld_sem, 16 * (1 if g == 0 else g + 2))
                b = g % NBUF
                nip_g = SIZES[g] + 16
                gp.dma_gather(
                    out_ap=dst_sb[:, b, : nip_g // 128 + 1],
                    in_ap=pair_view,
                    idxs_ap=idx_sb[:, IOFF[g] : IOFF[g + 1]],
                    num_idxs=nip_g,
                    num_idxs_reg=nip_g,
                    elem_size=2 * D,
                    elem_step=2 * D,
                    single_packet=False,
                    queue_num=g % N_QUEUES,
                ).then_inc(dma_sems[g % N_DMA_SEMS], 16)
            for k in range(N_DMA_SEMS):
                gp.wait_ge(dma_sems[k], 16 * ((NG - 1 - k) // N_DMA_SEMS + 1))

        @block.vector
        def _(dve):
            dve.wait_ge(ld_sem, 32)  # msk resident (2nd load)
            for g in range(NG):
                dve.wait_ge(dma_sems[g % N_DMA_SEMS], 16 * (g // N_DMA_SEMS + 1))
                lo, hi, mexp = lo_hi_mexp(g)
                dve.copy_predicated(lo, mexp, hi).then_inc(pred_sem, 1)

        @block.scalar
        def _(act):
            for g in range(NG):
                act.wait_ge(pred_sem, g + 1)
                if g >= NOUT:
                    act.wait_ge(store_sem, 16 * (g - NOUT + 1))
                lo, _, _ = lo_hi_mexp(g)
                act.copy(
                    cont_sb[:, g % NOUT, : SIZES[g] * D // 128].rearrange(
                        "p (c d) -> p c d", d=D
                    ),
                    lo,
                ).then_inc(act_sem, 1)

    nc.compile()
    return nc


def _get_nc():
    if "nc" not in _cached:
        _cached["nc"] = _build()
    return _cached["nc"]


# slot i (gather list position) <-> within-group position t: the gather
# writes entry i to dst[i % 128, i // 128], and partition p must hold
# positions p*ncols .. +ncols-1, so i = (t % ncols)*128 + (t // ncols).
_T_OF_SLOT = {
    ni: np.arange(ni).reshape(128, ni // 128).T.ravel() for ni in set(SIZES)
}


def make_in_maps(index: np.ndarray, weight: np.ndarray):
    import ml_dtypes

    idx_flat = np.ascontiguousarray(index, dtype=np.int64).reshape(-1)
    ext = np.ascontiguousarray(weight.T).astype(ml_dtypes.bfloat16)

    in_maps = []
    for c in range(N_CORES):
        v = idx_flat[c * N_CORE : (c + 1) * N_CORE]
        pair_idx = ((v >> 1) - 32768).astype(np.int16)  # [N_CORE]
        odd = (v & 1).astype(np.uint8)  # [N_CORE]

        idx_arr = np.empty((128, IOFF[NG]), dtype=np.int16)
        msk_arr = np.empty((128, MOFF[NG]), dtype=np.uint8)
        for g in range(NG):
            ni = SIZES[g]
            p_g = pair_idx[OFFS[g] : OFFS[g] + ni]
            o_g = odd[OFFS[g] : OFFS[g] + ni]
            slots = np.zeros(ni + 16, dtype=np.int16)  # trailing dummies = 0
            slots[:ni] = p_g[_T_OF_SLOT[ni]]
            # [16, icols] stripe (entry i at [i%16, i//16]), replicated 8x
            # down the partitions -- one copy per Q7 core pair
            stripe = slots.reshape(ICOLS_G[g], 16).T
            idx_arr[:, IOFF[g] : IOFF[g + 1]] = np.tile(stripe, (8, 1))
            # mask in dst layout: [p, c] = v&1 of position p*ncols+c
            msk_arr[:, MOFF[g] : MOFF[g + 1]] = o_g.reshape(128, ni // 128)
        in_maps.append({"idx": idx_arr, "msk": msk_arr, "ext": ext})
    return in_maps


def kernel(index: np.ndarray, weight: np.ndarray) -> np.ndarray:
    in_maps = make_in_maps(index, weight)
    nc = _get_nc()
    res = run_bass_kernel_spmd(nc, in_maps, core_ids=list(range(N_CORES)))
    outs = [np.asarray(r["out"]).astype(np.float32) for r in res.results]
    full = np.concatenate(outs, axis=0)  # [819200, 128]
    return full.reshape(index.shape[0], index.shape[1], D)
